# revision 1
# baseline (speedup 1.0000x reference)
"""Cosine cross-attention (B=4, L=2048, D=1024, H=16, dh=64, tau=0.07) on 8 trn2 cores.

Sharding: core = b*2 + g  (b in 0..3 data-parallel, g in 0..1 head-group of 8 heads).
Per core everything is computed feature-major ("T" = transposed, [feature, L]):
  QT = l2norm-by-head( wq.T @ xqT + bq )        [512, 2048]  (normalization via
      PE-broadcast of 1/||q|| and in-place DVE multiply)
  KT =                 wk.T @ xkT + bk          [512, 2048]  (its 1/||k||/tau goes
      into the per-partition scale of the exp activation)
  V  natural layout    (xvT.T chunks) @ wv      [2048, 512]  no bias (bv folded
      into a host-side output bias: softmax rows sum to 1)
  per head pair (m) / head (s):   S.T tile = KT_h.T-chunk.T @ QT-block  (K=64,
      auto row-tiled 64x128 so the two heads share the PE array; K itself is
      pre-scaled by rnk/tau so the exp needs no per-partition scale and one
      ACT call covers both heads)
  E.T = exp(S.T)  on ACT, psum->sbuf f32r
  OT  = [V | 1].T @ E.T  accumulated over Lk in PSUM -> row 64 is the softmax Z
  MT  = OT[0:64]/Z  via DVE reciprocal + PE-broadcast + in-place multiply
  OUT.T partial = wo-chunks.T @ MT-chunks       [1024, 2048]
Host: out[b] = (partial_g0 + partial_g1).T + (bo + bv @ Wo.T).

All matmuls run as float32r (tf32-like, 1 cycle/row at N>=256).
"""

import os

# some harnesses pin jax to cpu for the reference; this kernel needs the
# axon/neuron backend, so clear the pin before jax is first imported
if os.environ.get("JAX_PLATFORMS") == "cpu":
    del os.environ["JAX_PLATFORMS"]

import numpy as np

import concourse.bacc as bacc
import concourse.tile as tile
from concourse import mybir
from concourse.bass_utils import run_bass_kernel_spmd

P = 128
L = 2048
D = 1024
DO = 512  # per-core output dims of q/k/v projections (8 heads * 64)
TAU = 0.07
NLB = L // 512   # 4 blocks of 512 along L
NLK = L // 128   # 16 chunks of 128 along L (keys)
NM = DO // P     # 4 dout chunks (head pairs)
NKC = D // P     # 8 contraction chunks for projections

F32 = mybir.dt.float32
F32R = mybir.dt.float32r
BF16 = mybir.dt.bfloat16
EXP = mybir.ActivationFunctionType.Exp
SQRT = mybir.ActivationFunctionType.Sqrt
SQUARE = mybir.ActivationFunctionType.Square
MULT = mybir.AluOpType.mult

_CACHE = {}
VARIANT = None


def _emit(nc, prm, repeat=1, phases="abcd"):
    from contextlib import ExitStack
    with tile.TileContext(nc) as tc:
        if repeat > 1:
            with tc.For_i(0, repeat, 1):
                _emit_body(nc, tc, prm, phases)
        else:
            _emit_body(nc, tc, prm, phases)


def _emit_body(nc, tc, prm, phases="abcd"):
    from contextlib import ExitStack
    with ExitStack() as stack:
        const = stack.enter_context(tc.tile_pool(name="const", bufs=1))
        persist = stack.enter_context(tc.tile_pool(name="persist", bufs=1))
        normp = stack.enter_context(tc.tile_pool(name="normp", bufs=2))

        indt = const.tile([P, 2], F32R, tag="indt")
        nc.sync.dma_start(out=indt[:], in_=prm["indt"][:])
        ones8 = const.tile([P, 8], F32R, tag="ones8")
        nc.sync.dma_start(out=ones8[:], in_=prm["ones8"][:])
        selq = const.tile([8, NM, P], F32R, tag="selq")
        nc.sync.dma_start(out=selq[:], in_=prm["selq"][:])
        selz = const.tile([8, 8, 64], F32R, tag="selz")
        nc.sync.dma_start(out=selz[:], in_=prm["selz"][:])
        bq_t = const.tile([P, NM], F32, tag="bq")
        bk_t = const.tile([P, NM], F32, tag="bk")
        for m in range(NM):
            nc.sync.dma_start(out=bq_t[:, m], in_=prm["bq"][m * P:(m + 1) * P])
            nc.sync.dma_start(out=bk_t[:, m], in_=prm["bk"][m * P:(m + 1) * P])

        qt = [persist.tile([P, L], F32R, tag=f"qt{m}", name=f"qt{m}") for m in range(NM)]
        kt = [persist.tile([P, L], F32R, tag=f"kt{m}", name=f"kt{m}") for m in range(NM)]
        vg_all = persist.tile([P, NLK, 8, 65], F32R, tag="vg_all")
        vg = [vg_all[:, i] for i in range(NLK)]
        nq_all = persist.tile([8, L], F32R, tag="nq_all")
        nk_all = persist.tile([8, L], F32R, tag="nk_all")

        # ---------------- Phase A: projections ----------------
        with tc.tile_pool(name="wp", bufs=1) as wp, \
             tc.tile_pool(name="xp", bufs=3) as xp, \
             tc.tile_pool(name="sqp", bufs=1) as sqp, \
             tc.tile_pool(name="psA", bufs=1, space="PSUM") as psA, \
             tc.tile_pool(name="psN", bufs=1, space="PSUM") as psN, \
             tc.tile_pool(name="psBC", bufs=2, space="PSUM") as psBC:

            for kind in (("q", "k", "v") if "a" in phases else ()):
                w_d = prm["w" + kind]
                x_d = prm["x" + kind]
                wt = []
                for kc in range(NKC):
                    w_t = wp.tile([P, DO], F32R, tag=f"w{kc}")
                    nc.sync.dma_start(out=w_t[:], in_=w_d[kc * P:(kc + 1) * P, :])
                    wt.append(w_t)
                for lb in range(NLB):
                    pas = [psA.tile([P, 512], F32, tag=f"pa{j}", name=f"pa{j}") for j in range(NM)]
                    for kc2 in range(NKC // 2):
                        # paired-chunk load: one DMA brings two contraction chunks
                        x_t = xp.tile([P, 2, 512], F32R, tag="x")
                        nc.sync.dma_start(
                            out=x_t[:],
                            in_=x_d[2 * kc2 * P:(2 * kc2 + 2) * P,
                                    lb * 512:(lb + 1) * 512].rearrange(
                                        "(two p) i -> p two i", two=2))
                        for half in range(2):
                            kc = 2 * kc2 + half
                            xv = x_t[:, half, :]
                            if kind == "v":
                                for j in range(NM):
                                    nc.tensor.matmul(
                                        pas[j][:], lhsT=xv[:, j * P:(j + 1) * P], rhs=wt[kc][:],
                                        start=(kc == 0), stop=(kc == NKC - 1))
                            else:
                                for m in range(NM):
                                    nc.tensor.matmul(
                                        pas[m][:], lhsT=wt[kc][:, m * P:(m + 1) * P], rhs=xv,
                                        start=(kc == 0), stop=(kc == NKC - 1))
                    if kind == "v":
                        for j in range(NM):
                            lc = lb * 4 + j
                            nc.vector.tensor_copy(
                                out=vg[lc][:, :, 0:64],
                                in_=pas[j][:].rearrange("p (h d) -> p h d", h=8))
                            nc.vector.tensor_copy(out=vg[lc][:, :, 64], in_=ones8[:])
                    else:
                        b_t = bq_t if kind == "q" else bk_t
                        n_all = nq_all if kind == "q" else nk_all
                        for m in range(NM):
                            sl = slice(lb * 512, (lb + 1) * 512)
                            blk = (qt if kind == "q" else kt)[m][:, sl]
                            nc.vector.tensor_scalar_add(
                                out=blk, in0=pas[m][:], scalar1=b_t[:, m:m + 1])
                            sq_t = sqp.tile([P, 512], F32R, tag="sq")
                            # (x+b)^2 in one ACT op -- ScalarE is idle in phase A
                            nc.scalar.activation(out=sq_t[:], in_=pas[m][:], func=SQUARE,
                                                 bias=b_t[:, m:m + 1])
                            nqp = psN.tile([2, 512], F32, tag="nq")
                            nc.tensor.matmul(nqp[:], lhsT=indt[:], rhs=sq_t[:],
                                             start=True, stop=True)
                            nqb = normp.tile([2, 512], F32R, tag="nqb", bufs=1)
                            nc.vector.tensor_copy(out=nqb[:], in_=nqp[:])
                            nc.sync.dma_start(
                                out=n_all[2 * m:2 * m + 2, lb * 512:(lb + 1) * 512],
                                in_=nqb[:])

            # ---------------- Phase B: norms ----------------
            if "b" in phases:
                _emit_norms(nc, tc, normp, psBC, qt, kt, nq_all, nk_all, selq)

        # prefetch out-projection weights so phase D starts immediately
        wop = stack.enter_context(tc.tile_pool(name="wop", bufs=1))
        wot = []
        for kc in range(NM):
            w_t = wop.tile([P, D], F32R, tag=f"wo{kc}", name=f"wo{kc}")
            nc.sync.dma_start(out=w_t[:], in_=prm["wo"][kc * P:(kc + 1) * P, :])
            wot.append(w_t)

        # ---------------- Phase C: attention ----------------
        mtp = stack.enter_context(tc.tile_pool(name="mtp", bufs=1))
        mt = [mtp.tile([P, L], F32R, tag=f"mt{m}", name=f"mt{m}") for m in range(NM)]
        with tc.tile_pool(name="psS", bufs=2, space="PSUM") as psS, \
             tc.tile_pool(name="psOT", bufs=2, space="PSUM") as psOT, \
             tc.tile_pool(name="etp", bufs=4) as etp:
            for m in range(NM if "c" in phases else 0):
                zpack = normp.tile([8, 512], F32R, tag="zp", bufs=1)
                for lq in range(NLB):
                    ot0 = psOT.tile([65, 512], F32, tag="ot0")
                    ot1 = psOT.tile([65, 512], F32, tag="ot1")
                    for lk in range(NLK):
                        pss = psS.tile([P, 1024], F32, tag="pss")
                        # the two heads run in opposite PE array halves
                        for s in range(2):
                            base = s * 64
                            nc.tensor.matmul(
                                pss[:, s * 512:(s + 1) * 512],
                                lhsT=kt[m][base:base + 64, lk * P:(lk + 1) * P],
                                rhs=qt[m][base:base + 64, lq * 512:(lq + 1) * 512],
                                start=True, stop=True)
                        et = etp.tile([P, 1024], F32R, tag="et")
                        nc.scalar.activation(out=et[:], in_=pss[:], func=EXP)
                        _emit_pv(nc, vg, et, lk, ot0, ot1, m)
                    for s, ot in enumerate((ot0, ot1)):
                        nc.vector.tensor_copy(
                            out=mt[m][s * 64:s * 64 + 64, lq * 512:(lq + 1) * 512],
                            in_=ot[0:64, :])
                        zb = normp.tile([1, 512], F32R, tag="zb", bufs=1)
                        nc.vector.tensor_copy(out=zb[:], in_=ot[64:65, :])
                        r = s * 4 + lq
                        nc.sync.dma_start(out=zpack[r:r + 1, :], in_=zb[:])
                rz = zpack
                with nc.allow_low_precision(reason="f32r reciprocal, tf32 rounding is fine here"):
                    nc.vector.reciprocal(out=rz[:], in_=zpack[:])
                for s in range(2):
                    for lq in range(NLB):
                        r = s * 4 + lq
                        # borrow a pss slot for the Z broadcast (the score
                        # pipeline drains at the m boundary anyway)
                        bc = psS.tile([P, 1024], F32, tag="pss")
                        nc.tensor.matmul(bc[0:64, 0:512], lhsT=selz[:, r, :], rhs=rz[:],
                                         start=True, stop=True)
                        blk = mt[m][s * 64:s * 64 + 64, lq * 512:(lq + 1) * 512]
                        nc.vector.tensor_tensor(out=blk, in0=blk, in1=bc[0:64, 0:512],
                                                op=MULT)

        # ---------------- Phase D: output projection ----------------
        if "d" not in phases:
            ob0 = normp.tile([P, 512], F32, tag="dummyout")
            nc.vector.memset(ob0[:], 0.0)
            nc.sync.dma_start(out=prm["out_t"][0:P, 0:512], in_=ob0[:])
            return
        with tc.tile_pool(name="obp", bufs=4) as obp, \
             tc.tile_pool(name="psD", bufs=4, space="PSUM") as psD:
            for mo in range(D // P):
                for lb in range(NLB):
                    pd = psD.tile([P, 512], F32, tag="pd")
                    for kc in range(NM):
                        nc.tensor.matmul(pd[:], lhsT=wot[kc][:, mo * P:(mo + 1) * P],
                                         rhs=mt[kc][:, lb * 512:(lb + 1) * 512],
                                         start=(kc == 0), stop=(kc == NM - 1))
                    ob = obp.tile([P, 512], F32, tag="ob")
                    nc.vector.tensor_copy(out=ob[:], in_=pd[:])
                    nc.sync.dma_start(
                        out=prm["out_t"][mo * P:(mo + 1) * P, lb * 512:(lb + 1) * 512],
                        in_=ob[:])



def build_nc(repeat=1, phases="abcd"):
    key = (repeat, phases)
    if key in _CACHE:
        return _CACHE[key]
    nc = bacc.Bacc("TRN2", target_bir_lowering=False, debug=False, num_devices=8)
    prm = {}
    for name in ("xq", "xk", "xv"):
        prm[name] = nc.declare_dram_parameter(name, [D, L], F32R, isOutput=False)
    for name in ("wq", "wk", "wv"):
        prm[name] = nc.declare_dram_parameter(name, [D, DO], F32R, isOutput=False)
    prm["wo"] = nc.declare_dram_parameter("wo", [DO, D], F32R, isOutput=False)
    prm["bq"] = nc.declare_dram_parameter("bq", [DO], F32, isOutput=False)
    prm["bk"] = nc.declare_dram_parameter("bk", [DO], F32, isOutput=False)
    prm["indt"] = nc.declare_dram_parameter("indt", [P, 2], F32R, isOutput=False)
    prm["ones8"] = nc.declare_dram_parameter("ones8", [P, 8], F32R, isOutput=False)
    prm["selq"] = nc.declare_dram_parameter("selq", [8, NM, P], F32R, isOutput=False)
    prm["selz"] = nc.declare_dram_parameter("selz", [8, 8, 64], F32R, isOutput=False)
    prm["out_t"] = nc.declare_dram_parameter("out_t", [D, L], F32, isOutput=True)
    _emit(nc, prm, repeat=repeat, phases=phases)
    nc.compile()
    _CACHE[key] = nc
    return nc


def make_in_maps(q, k, v, Wq, bq, Wk, bk, Wv, bv, Wo, bo):
    B = q.shape[0]
    f32 = np.float32

    indt = np.zeros((P, 2), f32)
    indt[0:64, 0] = 1.0
    indt[64:128, 1] = 1.0
    ones8 = np.ones((P, 8), f32)
    selq = np.zeros((8, NM, P), f32)
    for m in range(NM):
        for j in range(P):
            selq[2 * m + j // 64, m, j] = 1.0
    selz = np.zeros((8, 8, 64), f32)
    for r in range(8):
        selz[r, r, :] = 1.0

    in_maps = []
    for b in range(B):
        for g in range(2):
            sl = slice(g * DO, (g + 1) * DO)
            in_maps.append({
                "xq": np.ascontiguousarray(q[b].T.astype(f32)),
                "xk": np.ascontiguousarray(k[b].T.astype(f32)),
                "xv": np.ascontiguousarray(v[b].T.astype(f32)),
                "wq": np.ascontiguousarray(Wq[sl, :].T.astype(f32)),
                "wk": np.ascontiguousarray(Wk[sl, :].T.astype(f32)),
                "wv": np.ascontiguousarray(Wv[sl, :].T.astype(f32)),
                "wo": np.ascontiguousarray(Wo[:, sl].T.astype(f32)),
                "bq": np.ascontiguousarray(bq[sl].astype(f32)),
                "bk": np.ascontiguousarray(bk[sl].astype(f32)),
                "indt": indt, "ones8": ones8, "selq": selq, "selz": selz,
            })
    return in_maps


def assemble(results, bv, Wo, bo):
    B = len(results) // 2
    bias = (bo + bv @ Wo.T).astype(np.float32)
    outs = []
    for b in range(B):
        part = results[2 * b]["out_t"] + results[2 * b + 1]["out_t"]
        outs.append(part.T + bias)
    return np.stack(outs).astype(np.float32)


def kernel(q, k, v, Wq, bq, Wk, bk, Wv, bv, Wo, bo):
    q, k, v = (np.asarray(t, np.float32) for t in (q, k, v))
    Wq, bq, Wk, bk, Wv, bv, Wo, bo = (
        np.asarray(t, np.float32) for t in (Wq, bq, Wk, bk, Wv, bv, Wo, bo))
    nc = build_nc()
    in_maps = make_in_maps(q, k, v, Wq, bq, Wk, bk, Wv, bv, Wo, bo)
    last_err = None
    for attempt in range(3):
        try:
            res = run_bass_kernel_spmd(nc, in_maps, core_ids=list(range(8)))
            return assemble(res.results, bv, Wo, bo)
        except Exception as e:  # transient NRT device errors: retry
            last_err = e
            import time as _time
            _time.sleep(2.0)
    raise last_err


def _emit_norms(nc, tc, normp, psBC, qt, kt, nq_all, nk_all, selq):
    with nc.allow_low_precision(reason="f32r norm chain, tf32 rounding fine"):
        nc.scalar.activation(out=nq_all[:], in_=nq_all[:], func=SQRT)
        nc.vector.tensor_scalar_max(out=nq_all[:], in0=nq_all[:], scalar1=1e-12)
        nc.vector.reciprocal(out=nq_all[:], in_=nq_all[:])
        nc.scalar.activation(out=nk_all[:], in_=nk_all[:], func=SQRT)
        # clamp at eps, then fold the softmax temperature into k's norm
        nc.vector.tensor_scalar_max(out=nk_all[:], in0=nk_all[:], scalar1=1e-12)
        nc.vector.tensor_scalar_mul(out=nk_all[:], in0=nk_all[:], scalar1=TAU)
        nc.vector.reciprocal(out=nk_all[:], in_=nk_all[:])

    # normalize Q and K in place via PE-broadcast of the row pair
    for which, r_all in (("q", nq_all), ("k", nk_all)):
        for m in range(NM):
            for lb in range(NLB):
                sl = slice(lb * 512, (lb + 1) * 512)
                bc = psBC.tile([P, 512], F32, tag="bcq")
                nc.tensor.matmul(bc[:], lhsT=selq[:, m, :], rhs=r_all[:, sl],
                                 start=True, stop=True)
                blk = (qt if which == "q" else kt)[m][:, sl]
                nc.vector.tensor_tensor(out=blk, in0=blk, in1=bc[:], op=MULT)


def _emit_pv(nc, vg, et, lk, ot0, ot1, m):
    nc.tensor.matmul(ot0[:], lhsT=vg[lk][:, 2 * m, :], rhs=et[:, 0:512],
                     start=(lk == 0), stop=(lk == NLK - 1), skip_group_check=True)
    nc.tensor.matmul(ot1[:], lhsT=vg[lk][:, 2 * m + 1, :], rhs=et[:, 512:1024],
                     start=(lk == 0), stop=(lk == NLK - 1), skip_group_check=True)



# revision 2
# speedup vs baseline: 1.1825x; 1.1825x over previous
"""Cosine cross-attention (B=4, L=2048, D=1024, H=16, dh=64, tau=0.07) on 8 trn2 cores.

Sharding: core = b*2 + g  (b in 0..3 data-parallel, g in 0..1 head-group of 8 heads).

v2 layout — engineered so the ACT engine (exp) is the only near-critical
resource and every other engine pipelines around it:

  Phase A (PE ~55us): q/k projections feature-major QT/KT [512, 2048] f32r.
    Norms fused per (m, lb) block: ACT square(+bias) -> PE indt-matmul
    (sum over the two heads' 64 partitions) -> ACT sqrt -> DVE max/recip ->
    PE K=2 broadcast matmul (sel2; k-side carries 1/tau) -> DVE multiply.
  Phase C (ACT-bound ~270us): per (lq, m): 16 lk chunks of
    scores S.T = KT-chunk.T @ QT-block (f32r, two heads in the two PE
    halves), exp on ACT -> ET bf16, then natural-layout PV in bf16:
    O[q,f] accumulated by 8 matmuls (M=128 q-chunk, N=65 = dh+Z-column)
    into column-packed PSUM accumulators. Drain: DVE reciprocal of Z,
    per-partition scale -> MN bf16, PE transpose (identity matmul) back to
    feature-major MT, all deferred so PE never stalls behind DVE.
    The V projection (bf16) is emitted inside (lq0, m0)'s lk loop, and the
    out-projection for each lq-block is emitted during lq+1 — both fill PE
    gaps while ACT streams exp.
  PSUM budget (8 banks): scores 2x2 + O-accum 2x1 + shared dt-ring 2.

Host: out[b] = (partial_g0 + partial_g1).T + (bo + bv @ Wo.T).
"""

import os

# some harnesses pin jax to cpu for the reference; this kernel needs the
# axon/neuron backend, so clear the pin before jax is first imported
if os.environ.get("JAX_PLATFORMS") == "cpu":
    del os.environ["JAX_PLATFORMS"]

import numpy as np

import concourse.bacc as bacc
import concourse.tile as tile
from concourse import mybir
from concourse.bass_utils import run_bass_kernel_spmd

P = 128
L = 2048
D = 1024
DO = 512  # per-core output dims of q/k/v projections (8 heads * 64)
TAU = 0.07
NLB = L // 512   # 4 blocks of 512 along L
NLK = L // 128   # 16 chunks of 128 along L (keys)
NM = DO // P     # 4 dout chunks (head pairs)
NKC = D // P     # 8 contraction chunks for projections

F32 = mybir.dt.float32
F32R = mybir.dt.float32r
BF16 = mybir.dt.bfloat16
EXP = mybir.ActivationFunctionType.Exp
SQRT = mybir.ActivationFunctionType.Sqrt
SQUARE = mybir.ActivationFunctionType.Square
MULT = mybir.AluOpType.mult

_CACHE = {}


DEBUG_TAPS = False


def _emit(nc, prm, repeat=1):
    with tile.TileContext(nc) as tc:
        if repeat > 1:
            with tc.For_i(0, repeat, 1):
                _emit_body(nc, tc, prm)
        else:
            _emit_body(nc, tc, prm)


def _emit_body(nc, tc, prm):
    from contextlib import ExitStack
    with ExitStack() as stack:
        const = stack.enter_context(tc.tile_pool(name="const", bufs=1))
        persist = stack.enter_context(tc.tile_pool(name="persist", bufs=1))

        # constants are DMA'd later (after the first projection block's
        # weights/activations) so the shared DMA engines start on the
        # critical-path loads immediately; tiles are declared here
        bq_t = const.tile([P, NM], F32, tag="bq")
        bk_t = const.tile([P, NM], F32, tag="bk")
        indt = const.tile([P, 2], BF16, tag="indt")
        sel2q = const.tile([2, P], F32R, tag="sel2q")
        sel2k = const.tile([2, P], F32R, tag="sel2k")
        ident = const.tile([P, P], BF16, tag="ident")

        def emit_const_dmas(step):
            # staggered behind the projection x-stream: only what the next
            # block needs, so the shared DMA engines stay on the critical path
            if step == 0:
                for m in range(NM):
                    nc.scalar.dma_start(out=bq_t[:, m],
                                        in_=prm["bq"][m * P:(m + 1) * P])
                    nc.scalar.dma_start(out=bk_t[:, m],
                                        in_=prm["bk"][m * P:(m + 1) * P])
                nc.scalar.dma_start(out=indt[:], in_=prm["indt"][:])
            elif step == 1:
                nc.scalar.dma_start(out=sel2q[:], in_=prm["sel2q"][:])
                nc.scalar.dma_start(out=sel2k[:], in_=prm["sel2k"][:])
            elif step == 2:
                nc.scalar.dma_start(out=ident[:], in_=prm["ident"][:])

        qt = [persist.tile([P, L], F32R, tag=f"qt{m}", name=f"qt{m}") for m in range(NM)]
        kt = [persist.tile([P, L], F32R, tag=f"kt{m}", name=f"kt{m}") for m in range(NM)]
        vg_all = persist.tile([P, NLK, 8, 65], BF16, tag="vg_all")
        vg = [vg_all[:, i] for i in range(NLK)]
        # softmax-Z ones column of [V | 1]
        nc.vector.memset(vg_all[:, :, :, 64], 1.0)

        dbgp2 = stack.enter_context(tc.tile_pool(name="dbg2", bufs=1)) \
            if DEBUG_TAPS else None

        wop = stack.enter_context(tc.tile_pool(name="wop", bufs=1))
        wot = []

        # V-projection inputs/weights, prefetched at phase-C open (ACT queue);
        # phase C consumes them with zero DMA on its critical path
        wvp = stack.enter_context(tc.tile_pool(name="wvp", bufs=1))
        xvp = stack.enter_context(tc.tile_pool(name="xvp", bufs=1))
        wvt = []
        xvt = []

        # ---------------- Phase A: q/k projections + pipelined norms -----------
        # The per-block norm chain (square -> partition-sum matmul -> sqrt ->
        # clamp/recip -> K=2 broadcast matmul -> multiply) spans four engines;
        # run it as a two-stage deferred pipeline so the PE never waits on it:
        # S0 (inline): DVE bias-add, GPSIMD square.  S1 (one block later):
        # norm matmuls + sqrt + recip.  S2 (two blocks later): broadcast +
        # multiply.  Flushes land between the next block's projection matmuls.
        from collections import deque
        with tc.tile_pool(name="wp", bufs=1) as wp, \
             tc.tile_pool(name="xp", bufs=3) as xp, \
             tc.tile_pool(name="sqp", bufs=2) as sqp, \
             tc.tile_pool(name="nrp", bufs=1) as nrp, \
             tc.tile_pool(name="psA", bufs=1, space="PSUM") as psA, \
             tc.tile_pool(name="psN", bufs=2, space="PSUM") as psN, \
             tc.tile_pool(name="psBC", bufs=2, space="PSUM") as psBC:
            pend1 = deque()
            pend2 = deque()

            def make_s2(nr, sel, dst, sl):
                def s2():
                    for m in range(NM):
                        bc = psBC.tile([P, 512], F32, tag="bc")
                        nc.tensor.matmul(bc[:], lhsT=sel[:],
                                         rhs=nr[:, m * 512:(m + 1) * 512],
                                         start=True, stop=True)
                        blk = dst[m][:, sl]
                        nc.vector.tensor_tensor(out=blk, in0=blk, in1=bc[:],
                                                op=MULT)
                return s2

            def make_s1(sqt, sel, dst, sl):
                def s1():
                    nr = nrp.tile([2, NM * 512], F32R, tag="nr")
                    for m in range(NM):
                        nqp = psN.tile([2, 512], F32, tag="nq")
                        nc.tensor.matmul(nqp[:], lhsT=indt[:], rhs=sqt[m][:],
                                         start=True, stop=True)
                        nrm = nr[:, m * 512:(m + 1) * 512]
                        nc.scalar.activation(out=nrm, in_=nqp[:], func=SQRT)
                        with nc.allow_low_precision(reason="f32r norm chain"):
                            nc.vector.tensor_scalar_max(out=nrm, in0=nrm,
                                                        scalar1=1e-12)
                            nc.vector.reciprocal(out=nrm, in_=nrm)
                    pend2.append(make_s2(nr, sel, dst, sl))
                return s1

            def flush():
                if pend2:
                    pend2.popleft()()
                if pend1:
                    pend1.popleft()()

            wts = {}
            for kind in ("q", "k"):
                wt = []
                for kc in range(NKC):
                    w_t = wp.tile([P, DO], F32R, tag=f"w{kind}{kc}")
                    wt.append(w_t)
                wts[kind] = wt
            for kc in range(NKC):
                nc.sync.dma_start(out=wts["q"][kc][:],
                                  in_=prm["wq"][kc * P:(kc + 1) * P, :])
            for kind in ("q", "k"):
                x_d = prm["x" + kind]
                b_t = bq_t if kind == "q" else bk_t
                sel = sel2q if kind == "q" else sel2k
                dst = qt if kind == "q" else kt
                wt = wts[kind]
                for lb in range(NLB):
                    sl = slice(lb * 512, (lb + 1) * 512)
                    pas = [psA.tile([P, 512], F32, tag=f"pa{j}", name=f"pa{j}")
                           for j in range(NM)]
                    for kc2 in range(NKC // 2):
                        # paired-chunk load: one DMA brings two contraction chunks
                        x_t = xp.tile([P, 2, 512], F32R, tag="x")
                        nc.sync.dma_start(
                            out=x_t[:],
                            in_=x_d[2 * kc2 * P:(2 * kc2 + 2) * P, sl].rearrange(
                                "(two p) i -> p two i", two=2))
                        for half in range(2):
                            kc = 2 * kc2 + half
                            xv = x_t[:, half, :]
                            for m in range(NM):
                                nc.tensor.matmul(
                                    pas[m][:], lhsT=wt[kc][:, m * P:(m + 1) * P],
                                    rhs=xv, start=(kc == 0), stop=(kc == NKC - 1))
                    if kind == "q":
                        if lb < 3:
                            emit_const_dmas(lb)
                        if lb == 2:
                            for kc in range(NM):
                                w_t = wop.tile([P, D], BF16, tag=f"wo{kc}",
                                               name=f"wo{kc}")
                                nc.scalar.dma_start(
                                    out=w_t[:],
                                    in_=prm["wo"][kc * P:(kc + 1) * P, :])
                                wot.append(w_t)
                        if lb == 3:
                            for kc in range(NKC):
                                nc.sync.dma_start(
                                    out=wts["k"][kc][:],
                                    in_=prm["wk"][kc * P:(kc + 1) * P, :])
                    # S0 first: the DVE bias-adds free the projection PSUM
                    # banks before the next block's matmuls need them
                    sqt = []
                    blks = []
                    for m in range(NM):
                        blk = dst[m][:, sl]
                        blks.append(blk)
                        nc.vector.tensor_scalar_add(
                            out=blk, in0=pas[m][:], scalar1=b_t[:, m:m + 1])
                    flush()
                    for m in range(NM):
                        sq_t = sqp.tile([P, 512], BF16, tag=f"sq{m}")
                        # (x+b)^2 on the otherwise-idle GPSIMD engine
                        nc.gpsimd.tensor_tensor(out=sq_t[:], in0=blks[m],
                                                in1=blks[m], op=MULT)
                        sqt.append(sq_t)
                    pend1.append(make_s1(sqt, sel, dst, sl))
            while pend1 or pend2:
                flush()

        # ---------------- Phase C: attention (ACT-bound, PE fills gaps) --------
        with tc.tile_pool(name="mtp", bufs=1) as mtp, \
             tc.tile_pool(name="etp", bufs=6) as etp, \
             tc.tile_pool(name="mnp", bufs=4) as mnp, \
             tc.tile_pool(name="rzp", bufs=4) as rzp, \
             tc.tile_pool(name="obp", bufs=2) as obp, \
             tc.tile_pool(name="psS", bufs=2, space="PSUM") as psS, \
             tc.tile_pool(name="psO", bufs=1, space="PSUM") as psO, \
             tc.tile_pool(name="psDT", bufs=2, space="PSUM") as psDT:
            mt = [mtp.tile([P, L], BF16, tag=f"mt{m}", name=f"mt{m}")
                  for m in range(NM)]
            # V-projection loads on the ACT DMA queue: the shared DMA engines
            # are idle once phase A drains, so these land before vproj needs
            # them
            for kc in range(NKC):
                w_t = wvp.tile([P, DO], BF16, tag=f"wv{kc}")
                nc.scalar.dma_start(out=w_t[:],
                                    in_=prm["wv"][kc * P:(kc + 1) * P, :])
                wvt.append(w_t)
            for lb in range(NLB):
                xv_t = xvp.tile([P, NKC, 512], BF16, tag=f"xv{lb}")
                nc.scalar.dma_start(
                    out=xv_t[:],
                    in_=prm["xv"][:, lb * 512:(lb + 1) * 512].rearrange(
                        "(c p) i -> p c i", p=P))
                xvt.append(xv_t)

            def emit_vproj(lb):
                # V projection for one 512-token block, natural layout, bf16
                for j in range(NM):
                    pv = psDT.tile([P, 512], F32, tag="dt")
                    for kc in range(NKC):
                        nc.tensor.matmul(
                            pv[:], lhsT=xvt[lb][:, kc, j * P:(j + 1) * P],
                            rhs=wvt[kc][:],
                            start=(kc == 0), stop=(kc == NKC - 1))
                    lc = lb * 4 + j
                    nc.vector.tensor_copy(
                        out=vg[lc][:, :, 0:64],
                        in_=pv[:].rearrange("p (h d) -> p h d", h=8))

            pending = []

            def make_tp(m, lq, mn):
                def tp():
                    # transpose MN [q, f] back to feature-major MT via PE
                    psT = psDT.tile([P, 512], BF16, tag="dt")
                    for s in range(2):
                        for c in range(4):
                            nc.tensor.transpose(
                                psT[s * 64:(s + 1) * 64, c * P:(c + 1) * P],
                                mn[:, (s * 4 + c) * 64:(s * 4 + c + 1) * 64],
                                ident[:])
                    nc.vector.tensor_copy(
                        out=mt[m][:, lq * 512:(lq + 1) * 512], in_=psT[:])
                return tp

            def make_op(mo, lq):
                def op():
                    pd = psDT.tile([P, 512], F32, tag="dt")
                    for kc in range(NM):
                        nc.tensor.matmul(
                            pd[:], lhsT=wot[kc][:, mo * P:(mo + 1) * P],
                            rhs=mt[kc][:, lq * 512:(lq + 1) * 512],
                            start=(kc == 0), stop=(kc == NM - 1))
                    ob = obp.tile([P, 512], BF16, tag="ob")
                    nc.vector.tensor_copy(out=ob[:], in_=pd[:])
                    nc.sync.dma_start(
                        out=prm["out_t"][mo * P:(mo + 1) * P,
                                         lq * 512:(lq + 1) * 512],
                        in_=ob[:])
                return op

            for lq in range(NLB):
                for m in range(NM):
                    oa = None
                    for lk in range(NLK):
                        if lq == 0 and m == 0 and lk % 4 == 0:
                            emit_vproj(lk // 4)
                        if lk in (2, 6, 10, 14) and pending:
                            pending.pop(0)()
                        pss = psS.tile([P, 1024], F32, tag="pss")
                        # the two heads run in opposite PE array halves
                        for s in range(2):
                            base = s * 64
                            nc.tensor.matmul(
                                pss[:, s * 512:(s + 1) * 512],
                                lhsT=kt[m][base:base + 64, lk * P:(lk + 1) * P],
                                rhs=qt[m][base:base + 64, lq * 512:(lq + 1) * 512],
                                start=True, stop=True)
                        et = etp.tile([P, 1024], BF16, tag="et")
                        nc.scalar.activation(out=et[:], in_=pss[:], func=EXP)
                        if DEBUG_TAPS and lq == 0 and m == 0 and lk == 0:
                            tps = dbgp2.tile([P, 1024], F32, tag="dbg_pss")
                            nc.vector.tensor_copy(out=tps[:], in_=pss[:])
                            nc.sync.dma_start(out=prm["dbg_pss"][:], in_=tps[:])
                            tpe = dbgp2.tile([P, 1024], F32, tag="dbg_et")
                            nc.vector.tensor_copy(out=tpe[:], in_=et[:])
                            nc.sync.dma_start(out=prm["dbg_et"][:], in_=tpe[:])
                        if lk == 0:
                            oa = [psO.tile([P, 260], F32, tag=f"oa{s}",
                                           name=f"oa{s}") for s in range(2)]
                        for s in range(2):
                            for c in range(4):
                                # start only on the bank's first matmul: a
                                # start marks the whole 2KB zero-region
                                # pending-zero, so later first-touches of the
                                # other column slices overwrite (not
                                # accumulate) as intended
                                nc.tensor.matmul(
                                    oa[s][:, c * 65:(c + 1) * 65],
                                    lhsT=et[:, s * 512 + c * P:s * 512 + (c + 1) * P],
                                    rhs=vg[lk][:, 2 * m + s, :],
                                    start=(lk == 0 and c == 0),
                                    stop=(lk == NLK - 1),
                                    skip_group_check=True)
                    # drain: 1/Z then per-partition scale into MN (natural [q, f])
                    mn = mnp.tile([P, 512], BF16, tag="mn")
                    for s in range(2):
                        rz = rzp.tile([P, 4], F32, tag="rz")
                        with nc.allow_low_precision(reason="softmax Z reciprocal"):
                            nc.vector.reciprocal(out=rz[:], in_=oa[s][:, 64:260:65])
                        for c in range(4):
                            nc.vector.tensor_scalar_mul(
                                out=mn[:, (s * 4 + c) * 64:(s * 4 + c + 1) * 64],
                                in0=oa[s][:, c * 65:c * 65 + 64],
                                scalar1=rz[:, c:c + 1])
                    if DEBUG_TAPS and lq == 0 and m == 0:
                        toa = dbgp2.tile([P, 260], F32, tag="dbg_oa")
                        nc.vector.tensor_copy(out=toa[:], in_=oa[0][:])
                        nc.sync.dma_start(out=prm["dbg_oa"][:], in_=toa[:])
                        tmn = dbgp2.tile([P, 512], F32, tag="dbg_mn")
                        nc.vector.tensor_copy(out=tmn[:], in_=mn[:])
                        nc.sync.dma_start(out=prm["dbg_mn"][:], in_=tmn[:])
                    pending.append(make_tp(m, lq, mn))
                    if m == NM - 1:
                        for mo in range(D // P):
                            pending.append(make_op(mo, lq))
            while pending:
                pending.pop(0)()

            if DEBUG_TAPS:
                with tc.tile_pool(name="dbgp", bufs=1) as dbgp:
                    for nm, t in (("dbg_qt", qt[0]), ("dbg_kt", kt[0])):
                        tmp = dbgp.tile([P, L], F32, tag=nm)
                        nc.vector.tensor_copy(out=tmp[:], in_=t[:])
                        nc.sync.dma_start(out=prm[nm][:], in_=tmp[:])
                    tmpv = dbgp.tile([P, 8, 65], F32, tag="dbg_vg")
                    nc.vector.tensor_copy(out=tmpv[:], in_=vg[0][:])
                    nc.sync.dma_start(out=prm["dbg_vg"][:], in_=tmpv[:])
                    tmpm = dbgp.tile([P, L], F32, tag="dbg_mt")
                    nc.vector.tensor_copy(out=tmpm[:], in_=mt[0][:])
                    nc.sync.dma_start(out=prm["dbg_mt"][:], in_=tmpm[:])


def build_nc(repeat=1):
    key = repeat
    if key in _CACHE:
        return _CACHE[key]
    nc = bacc.Bacc("TRN2", target_bir_lowering=False, debug=False, num_devices=8)
    prm = {}
    for name in ("xq", "xk"):
        prm[name] = nc.declare_dram_parameter(name, [D, L], F32R, isOutput=False)
    prm["xv"] = nc.declare_dram_parameter("xv", [D, L], BF16, isOutput=False)
    for name in ("wq", "wk"):
        prm[name] = nc.declare_dram_parameter(name, [D, DO], F32R, isOutput=False)
    prm["wv"] = nc.declare_dram_parameter("wv", [D, DO], BF16, isOutput=False)
    prm["wo"] = nc.declare_dram_parameter("wo", [DO, D], BF16, isOutput=False)
    prm["bq"] = nc.declare_dram_parameter("bq", [DO], F32, isOutput=False)
    prm["bk"] = nc.declare_dram_parameter("bk", [DO], F32, isOutput=False)
    prm["indt"] = nc.declare_dram_parameter("indt", [P, 2], BF16, isOutput=False)
    prm["sel2q"] = nc.declare_dram_parameter("sel2q", [2, P], F32R, isOutput=False)
    prm["sel2k"] = nc.declare_dram_parameter("sel2k", [2, P], F32R, isOutput=False)
    prm["ident"] = nc.declare_dram_parameter("ident", [P, P], BF16, isOutput=False)
    prm["out_t"] = nc.declare_dram_parameter("out_t", [D, L], BF16, isOutput=True)
    if DEBUG_TAPS:
        for nm in ("dbg_qt", "dbg_kt", "dbg_mt"):
            prm[nm] = nc.declare_dram_parameter(nm, [P, L], F32, isOutput=True)
        prm["dbg_vg"] = nc.declare_dram_parameter("dbg_vg", [P, 8, 65], F32,
                                                  isOutput=True)
        prm["dbg_pss"] = nc.declare_dram_parameter("dbg_pss", [P, 1024], F32,
                                                   isOutput=True)
        prm["dbg_et"] = nc.declare_dram_parameter("dbg_et", [P, 1024], F32,
                                                  isOutput=True)
        prm["dbg_oa"] = nc.declare_dram_parameter("dbg_oa", [P, 260], F32,
                                                  isOutput=True)
        prm["dbg_mn"] = nc.declare_dram_parameter("dbg_mn", [P, 512], F32,
                                                  isOutput=True)
    _emit(nc, prm, repeat=repeat)
    nc.compile()
    _CACHE[key] = nc
    return nc


def _bf16(a):
    import ml_dtypes
    return np.ascontiguousarray(a.astype(ml_dtypes.bfloat16))


def make_in_maps(q, k, v, Wq, bq, Wk, bk, Wv, bv, Wo, bo):
    B = q.shape[0]
    f32 = np.float32

    indt = np.zeros((P, 2), f32)
    indt[0:64, 0] = 1.0
    indt[64:128, 1] = 1.0
    indt = _bf16(indt)
    sel2q = np.zeros((2, P), f32)
    sel2q[0, 0:64] = 1.0
    sel2q[1, 64:128] = 1.0
    sel2k = sel2q * (1.0 / TAU)
    ident = np.eye(P, dtype=f32)

    in_maps = []
    for b in range(B):
        for g in range(2):
            sl = slice(g * DO, (g + 1) * DO)
            in_maps.append({
                "xq": np.ascontiguousarray(q[b].T.astype(f32)),
                "xk": np.ascontiguousarray(k[b].T.astype(f32)),
                "xv": _bf16(v[b].T),
                "wq": np.ascontiguousarray(Wq[sl, :].T.astype(f32)),
                "wk": np.ascontiguousarray(Wk[sl, :].T.astype(f32)),
                "wv": _bf16(Wv[sl, :].T),
                "wo": _bf16(Wo[:, sl].T),
                "bq": np.ascontiguousarray(bq[sl].astype(f32)),
                "bk": np.ascontiguousarray(bk[sl].astype(f32)),
                "indt": indt, "sel2q": sel2q, "sel2k": sel2k,
                "ident": _bf16(ident),
            })
    return in_maps


def assemble(results, bv, Wo, bo):
    B = len(results) // 2
    bias = (bo + bv @ Wo.T).astype(np.float32)
    outs = []
    for b in range(B):
        part = (results[2 * b]["out_t"].astype(np.float32)
                + results[2 * b + 1]["out_t"].astype(np.float32))
        outs.append(part.T + bias)
    return np.stack(outs).astype(np.float32)


def kernel(q, k, v, Wq, bq, Wk, bk, Wv, bv, Wo, bo):
    q, k, v = (np.asarray(t, np.float32) for t in (q, k, v))
    Wq, bq, Wk, bk, Wv, bv, Wo, bo = (
        np.asarray(t, np.float32) for t in (Wq, bq, Wk, bk, Wv, bv, Wo, bo))
    nc = build_nc()
    in_maps = make_in_maps(q, k, v, Wq, bq, Wk, bk, Wv, bv, Wo, bo)
    last_err = None
    for attempt in range(3):
        try:
            res = run_bass_kernel_spmd(nc, in_maps, core_ids=list(range(8)))
            return assemble(res.results, bv, Wo, bo)
        except Exception as e:  # transient NRT device errors: retry
            last_err = e
            import time as _time
            _time.sleep(2.0)
    raise last_err
